# revision 19
# baseline (speedup 1.0000x reference)
"""Trainium2 Bass kernel for nn_BaseLinearSSM (chunked formulation).

y[b,t] = Re(C @ x_{t+1}) + D @ u[b,t] + bias,  x_{t+1} = A x_t + B u_t  (complex A,B,C)

Strategy (chunk length L=8, NK=T/L=256 chunks):
  Host (fp64): eigendecompose A = V diag(w) V^-1, Bt = V^-1 B, Ct = C V.
  Precompute:
    Pt_j = diag(w^(L-1-j)) Bt          [N,IN]  (chunk input aggregation)
    Qt_j = Ct diag(w^(j+1))            [OUT,N] (chunk boundary -> outputs)
    K_d  = Re(C A^d B), K_0 += D       [OUT,IN] real (within-chunk causal conv)
  Device (per core, batch-sharded 2 of 16; fp16 data, fp32 PSUM/scan state):
    phase 1: vt_k = sum_j Pt_j u_{kL+j}                    (matmuls, PSUM)
    phase 2: S_k = w^L S_{k-1} + vt_k  via modulate/scan/demodulate on the
             CHUNK axis only (T/L columns -> 1/8 the DVE work of a full scan);
             demod written with a one-chunk shift so S_shift[k] = beta_k =
             state at chunk start (col k=0 memset to 0 per batch element)
    phase 3: y_{kL+j} = Re(Qt_j beta_k) + sum_d K_d u_{kL+j-d}  (matmuls)
  Time is laid out (j, b, k) so every matmul has 512 contiguous columns.
  Phase 3 runs in two waves (j0..5, j6..7) with the boundary matmuls ordered
  m-outer, so the tensor engine only needs the last S tiles at the very end
  of wave A (phase-2 tail hidden behind conv + earlier-m matmuls).
  Input DMA is split over the two HWDGE rings (sync + scalar queues).
  Host shards u, permutes layouts, gathers y, adds bias.
"""

import sys

import numpy as np

if "/opt/trn_rl_repo" not in sys.path:
    sys.path.insert(0, "/opt/trn_rl_repo")

BATCH, T, IN, OUT, N = 16, 2048, 128, 128, 512
NCORES = 8
BLOCAL = BATCH // NCORES   # 2
L = 8                      # chunk length
NK = T // L                # 256 chunks per batch element
NKB = BLOCAL * NK          # 512 chunk-columns per core (b-major)
NT = N // 128              # 4 partition tiles over the state dim
COLS = BLOCAL * T          # 4096

# blob (fp16) layout / DMA piece order:
#   sync queue:   u | Pt0 | Pt1 | Pt2 | Pt3
#   scalar queue: K | tr0 | tr1 | tr2 | tr3 | (deferred) Qt
# with tr_m = ck2 | sk2 | rho2.  Qt's dma_start is issued mid-phase-1 so its
# 2 MB does not steal HBM bandwidth from the phase-1-critical pieces.
UW = L * NKB               # 4096
KW = L * 128               # 1024
PW = 2 * L * 128           # 2048 per m
TW = 2 * NKB               # 1024 per m (cos+sin)
RW = NKB                   # 512 per m (rho, col NK zeroed)
QW = L * 2 * NT * 128      # 8192
TRW = TW + RW              # 1536 per m
W16 = UW + NT * PW + KW + NT * TRW + QW  # 27648

LAST_RESULT = None
_NC_CACHE = None


def _build_nc():
    from concourse import bass, mybir
    from concourse import tile

    f32 = mybir.dt.float32
    f16 = mybir.dt.float16
    op = mybir.AluOpType

    nc = bass.Bass("TRN2", target_bir_lowering=False, debug=False)

    blob = nc.dram_tensor("blob", [128, W16], f16, kind="ExternalInput")
    yout = nc.dram_tensor("y", [OUT, COLS], f32, kind="ExternalOutput")

    with tile.TileContext(nc) as tc:
        with (
            tc.tile_pool(name="const", bufs=1) as cpool,
            tc.tile_pool(name="vsb", bufs=2) as vpool,
            tc.tile_pool(name="tmp", bufs=2) as tpool,
            tc.tile_pool(name="gz", bufs=2) as gpool,
            tc.tile_pool(name="S", bufs=1) as spool,
            tc.tile_pool(name="ysb", bufs=4) as ypool_sb,
            tc.tile_pool(name="ps", bufs=1, space="PSUM") as pspool,
        ):
            b16 = cpool.tile([128, W16], f16)
            o = [0]

            def take(w):
                s = b16[:, o[0]:o[0] + w]
                o[0] += w
                return s

            u_jk = take(UW)
            ptT = [[[None] * L for _ in range(2)] for _ in range(NT)]
            for m in range(NT):
                for ri in range(2):
                    for j in range(L):
                        ptT[m][ri][j] = take(128)
            ktT = [take(128) for _ in range(L)]
            ck2 = [None] * NT
            sk2 = [None] * NT
            rho2 = [None] * NT
            for m in range(NT):
                ck2[m] = take(NKB)
                sk2[m] = take(NKB)
                rho2[m] = take(NKB)
            qtT = [[[None] * NT for _ in range(2)] for _ in range(L)]
            for j in range(L):
                for ri in range(2):
                    for m in range(NT):
                        qtT[j][ri][m] = take(128)
            assert o[0] == W16

            # sync queue: u, then each Pt piece (phase-1 critical path)
            nc.sync.dma_start(b16[:, 0:UW], blob[:, 0:UW])
            for m in range(NT):
                lo, hi = UW + m * PW, UW + (m + 1) * PW
                nc.sync.dma_start(b16[:, lo:hi], blob[:, lo:hi])
            # scalar queue: K and the first table piece up front; the rest
            # is triggered mid-phase-1 (staggered) so the phase-1-critical
            # Pt pieces get the HBM bandwidth first
            a = UW + NT * PW
            nc.scalar.dma_start(b16[:, a:a + KW], blob[:, a:a + KW])

            def tr_dma(m):
                lo, hi = a + KW + m * TRW, a + KW + (m + 1) * TRW
                nc.scalar.dma_start(b16[:, lo:hi], blob[:, lo:hi])

            tr_dma(0)

            # PE warm-up: ~12 dependency-free matmuls on scratch run during
            # the DMA head, flipping the HAM clock gate to 8/8 (2.4 GHz)
            # before phase 1 issues. Output is discarded.
            wsc = cpool.tile([128, NKB], f16)
            nc.vector.memset(wsc[:], 0.0)
            wp = pspool.tile([128, NKB], f32, tag="vt0", bufs=1, name="warm")
            for wi in range(12):
                nc.tensor.matmul(wp[:], wsc[:, :128], wsc[:],
                                 start=(wi == 0), stop=(wi == 11))

            Sr_t = [None] * NT
            Si_t = [None] * NT
            v_t = [None] * NT

            def phase1(m):
                v_sb = [None, None]
                for ri in range(2):
                    vt = pspool.tile([128, NKB], f32, tag=f"vt{ri}", bufs=1,
                                     name=f"vt{ri}")
                    for j in range(L):
                        nc.tensor.matmul(
                            vt[:], ptT[m][ri][j], u_jk[:, j * NKB:(j + 1) * NKB],
                            start=(j == 0), stop=(j == L - 1),
                        )
                    v_sb[ri] = vpool.tile([128, NKB], f16, tag=f"v{ri}m{m}",
                                          name=f"v{ri}m{m}", bufs=1)
                    nc.scalar.copy(v_sb[ri][:], vt[:])
                    if m < NT - 1 and ri == 1:
                        tr_dma(m + 1)  # next m's tables
                if m == 2:
                    # deferred: Qt's 2 MB rides the scalar queue only after
                    # the phase-1-critical DMA pieces have landed
                    nc.scalar.dma_start(b16[:, W16 - QW:W16],
                                        blob[:, W16 - QW:W16])
                v_t[m] = v_sb

            # phase 2 for a pair of m's with cross-m interleaved issue: the
            # in-order DVE fills one chain's dependency bubbles with the
            # sibling chain's ops. GpSimd carries the imag-modulate for all
            # but the last m (whose chain gates the final boundary matmuls).
            def phase2_pair(ms):
                tl = {}
                for m in ms:
                    for nm in ("t1", "t2", "t3", "t4", "t5", "t6", "t7", "t8"):
                        tl[nm, m] = tpool.tile([128, NKB], f16, tag=f"{nm}_{m}",
                                               name=f"{nm}_{m}", bufs=1)
                    for nm in ("gr", "gi", "zr", "zi"):
                        tl[nm, m] = gpool.tile([128, NKB], f16, tag=f"{nm}_{m}",
                                               name=f"{nm}_{m}", bufs=1)
                for m in ms:
                    vr, vi = v_t[m]
                    nc.vector.tensor_tensor(tl["t1", m][:], ck2[m], vr[:], op=op.mult)
                    nc.vector.tensor_tensor(tl["t2", m][:], sk2[m], vi[:], op=op.mult)
                    ie = nc.vector if m == NT - 1 else nc.gpsimd
                    ie.tensor_tensor(tl["t3", m][:], ck2[m], vi[:], op=op.mult)
                    ie.tensor_tensor(tl["t4", m][:], sk2[m], vr[:], op=op.mult)
                for m in ms:
                    nc.vector.tensor_tensor(tl["gr", m][:], tl["t1", m][:],
                                            tl["t2", m][:], op=op.add)
                    ie = nc.vector if m == NT - 1 else nc.gpsimd
                    ie.tensor_tensor(tl["gi", m][:], tl["t3", m][:],
                                     tl["t4", m][:], op=op.subtract)
                for m in ms:
                    nc.vector.tensor_tensor_scan(
                        tl["zr", m][:], rho2[m], tl["gr", m][:], 0.0,
                        op0=op.mult, op1=op.add)
                for m in ms:
                    nc.vector.tensor_tensor_scan(
                        tl["zi", m][:], rho2[m], tl["gi", m][:], 0.0,
                        op0=op.mult, op1=op.add)
                for m in ms:
                    nc.vector.tensor_tensor(tl["t5", m][:], ck2[m],
                                            tl["zr", m][:], op=op.mult)
                    nc.vector.tensor_tensor(tl["t6", m][:], sk2[m],
                                            tl["zi", m][:], op=op.mult)
                    nc.vector.tensor_tensor(tl["t7", m][:], sk2[m],
                                            tl["zr", m][:], op=op.mult)
                    nc.vector.tensor_tensor(tl["t8", m][:], ck2[m],
                                            tl["zi", m][:], op=op.mult)
                for m in ms:
                    Sr = spool.tile([128, NKB], f16, tag=f"Sr{m}", name=f"Sr{m}")
                    Si = spool.tile([128, NKB], f16, tag=f"Si{m}", name=f"Si{m}")
                    nc.gpsimd.memset(Sr[:, 0:1], 0.0)
                    nc.gpsimd.memset(Sr[:, NK:NK + 1], 0.0)
                    nc.gpsimd.memset(Si[:, 0:1], 0.0)
                    nc.gpsimd.memset(Si[:, NK:NK + 1], 0.0)
                    se = nc.vector if m == NT - 1 else nc.gpsimd
                    for b in range(BLOCAL):
                        a0 = b * NK
                        nc.vector.tensor_tensor(
                            Sr[:, a0 + 1:a0 + NK], tl["t5", m][:, a0:a0 + NK - 1],
                            tl["t6", m][:, a0:a0 + NK - 1], op=op.subtract,
                        )
                        se.tensor_tensor(
                            Si[:, a0 + 1:a0 + NK], tl["t7", m][:, a0:a0 + NK - 1],
                            tl["t8", m][:, a0:a0 + NK - 1], op=op.add,
                        )
                    Sr_t[m], Si_t[m] = Sr, Si

            phase1(0)
            phase1(1)
            phase2_pair((0, 1))
            phase1(2)
            phase1(3)
            phase2_pair((2, 3))

            # phase 3, two waves (j0..3, j4..7): conv first (only needs u/K),
            # then boundary matmuls m-outer so S(m) is consumed in completion
            # order. convB for j4/j5 is issued before wave A's m=3 batch to
            # keep the PE streaming while the last S tiles finish.
            yps = {}

            def conv(j):
                yps[j] = pspool.tile([128, NKB], f32, tag="y", bufs=6,
                                     name=f"y{j}")
                for d in range(j + 1):
                    nc.tensor.matmul(
                        yps[j][:], ktT[d],
                        u_jk[:, (j - d) * NKB:(j - d + 1) * NKB],
                        start=(d == 0), stop=False, skip_group_check=True,
                    )

            def bnd(j, m):
                nc.tensor.matmul(
                    yps[j][:], qtT[j][0][m], Sr_t[m][:], start=False,
                    stop=False, skip_group_check=True,
                )
                nc.tensor.matmul(
                    yps[j][:], qtT[j][1][m], Si_t[m][:], start=False,
                    stop=(m == NT - 1), skip_group_check=True,
                )

            def evict(j, eng):
                ysb = ypool_sb.tile([128, NKB], f32, tag="ysb")
                eng.copy(ysb[:], yps[j][:]) if eng is nc.scalar else \
                    eng.tensor_copy(ysb[:], yps[j][:])
                nc.sync.dma_start(yout[:, j * NKB:(j + 1) * NKB], ysb[:])

            for j in range(4):
                conv(j)
            for m in range(NT - 1):
                for j in range(4):
                    bnd(j, m)
            conv(4)
            conv(5)
            for j in range(4):
                bnd(j, NT - 1)
            for j in range(4):
                evict(j, nc.scalar)
            conv(6)
            conv(7)
            for m in range(NT):
                for j in range(4, L):
                    bnd(j, m)
            evict(4, nc.scalar)
            evict(5, nc.scalar)
            evict(6, nc.scalar)
            evict(7, nc.vector)

    _legalize_multi_waits(nc)
    return nc


def _legalize_multi_waits(nc):
    """This walrus build accepts a single sync wait per instruction; split
    any multi-wait instruction into same-engine single-wait NoOps + the
    original carrying the last wait (program order chains them)."""
    import bass_rust
    from concourse import mybir

    uid = [0]
    for fn in nc.m.functions:
        for bb in fn.blocks:
            insts = bb.instructions
            new = []
            changed = False
            for inst in insts:
                si = inst.sync_info
                if si is not None and len(si.on_wait) > 1:
                    waits = list(si.on_wait)
                    for w in waits[:-1]:
                        uid[0] += 1
                        new.append(mybir.InstNoOp(
                            name=f"mwsplit-{uid[0]}",
                            engine=inst.engine,
                            ins=[], outs=[],
                            sync_info=bass_rust.SyncInfo(on_wait=[w], on_update=[]),
                        ))
                    inst.sync_info = bass_rust.SyncInfo(
                        on_wait=[waits[-1]], on_update=list(si.on_update)
                    )
                    changed = True
                new.append(inst)
            if changed:
                bb.instructions = new


def _host_prep(A_re, A_im, B_re, B_im, C_re, C_im, D_w):
    """fp64 eigendecomposition + chunked-formulation weight/table layouts.
    Returns shared fp16 tail of the blob: [128, W16 - UW]."""
    A = A_re.astype(np.float64) + 1j * A_im.astype(np.float64)
    w, V = np.linalg.eig(A)
    Vinv = np.linalg.inv(V)
    Bt = Vinv @ (B_re.astype(np.float64) + 1j * B_im.astype(np.float64))
    Ct = (C_re.astype(np.float64) + 1j * C_im.astype(np.float64)) @ V

    Pt = np.stack([(w ** (L - 1 - j))[:, None] * Bt for j in range(L)])  # [L,N,IN]
    Qt = np.stack([Ct * (w ** (j + 1))[None, :] for j in range(L)])      # [L,OUT,N]
    K = np.empty((L, OUT, IN))
    Ad = np.eye(N, dtype=complex)
    Bc = B_re.astype(np.float64) + 1j * B_im.astype(np.float64)
    Cc = C_re.astype(np.float64) + 1j * C_im.astype(np.float64)
    for d in range(L):
        K[d] = (Cc @ Ad @ Bc).real
        Ad = A @ Ad
    K[0] += D_w.astype(np.float64)

    wL = w ** L
    rhoL = np.abs(wL)
    phi = np.angle(wL)
    kk = np.arange(NK)
    cosk = np.cos(np.outer(phi, kk + 1))  # [N, NK]
    sink = np.sin(np.outer(phi, kk + 1))

    parts = []
    for m in range(NT):
        sl = slice(m * 128, (m + 1) * 128)
        for Pp in (Pt.real, Pt.imag):
            for j in range(L):
                parts.append(np.ascontiguousarray(Pp[j].T[:, sl]))  # [IN, 128]
    for d in range(L):
        parts.append(np.ascontiguousarray(K[d].T))  # [IN, OUT]
    for m in range(NT):
        sl = slice(m * 128, (m + 1) * 128)
        parts.append(np.tile(cosk[sl], (1, BLOCAL)))  # [128, NKB]
        parts.append(np.tile(sink[sl], (1, BLOCAL)))
        rb = np.broadcast_to(rhoL[sl][:, None], (128, NKB)).copy()
        rb[:, NK] = 0.0  # reset scan state at second batch element
        parts.append(rb)
    for j in range(L):
        for Qp in (Qt[j].real, -Qt[j].imag):
            QT = np.ascontiguousarray(Qp.T)  # [N, OUT]
            for m in range(NT):
                parts.append(QT[m * 128:(m + 1) * 128])
    shared = np.concatenate(parts, axis=1).astype(np.float16)
    assert shared.shape == (128, W16 - UW)
    return shared


def _ensure_axon_hooks():
    """Provide antenv.axon_hooks if the image lacks it (needed only for
    trace=True NTFF profiling; run path works without)."""
    import types
    try:
        from antenv import axon_hooks  # noqa: F401
        return
    except ImportError:
        pass
    try:
        import antenv
        mod = types.ModuleType("antenv.axon_hooks")
        _hook = [None]
        mod.set_axon_ntff_profile_hook = lambda h: _hook.__setitem__(0, h)
        mod.get_axon_ntff_profile_hook = lambda: _hook[0]
        sys.modules["antenv.axon_hooks"] = mod
        antenv.axon_hooks = mod
        if "/root/.axon_site" not in sys.path:
            sys.path.insert(0, "/root/.axon_site")
        from trn_agent_boot.trn_boot import _ntff_profile_via_ctypes
        h = _ntff_profile_via_ctypes("/opt/axon/libaxon_pjrt.so")
        if h is not None:
            mod.set_axon_ntff_profile_hook(h)
    except Exception:
        pass


def kernel(u, A_re, A_im, B_re, B_im, C_re, C_im, D_w, output_bias):
    global LAST_RESULT, _NC_CACHE
    from concourse import bass_utils

    _ensure_axon_hooks()

    u = np.asarray(u, dtype=np.float32)
    shared = _host_prep(
        np.asarray(A_re), np.asarray(A_im), np.asarray(B_re), np.asarray(B_im),
        np.asarray(C_re), np.asarray(C_im), np.asarray(D_w)
    )

    if _NC_CACHE is None:
        _NC_CACHE = _build_nc()
    nc = _NC_CACHE

    in_maps = []
    for c in range(NCORES):
        up = u[BLOCAL * c:BLOCAL * (c + 1)]           # [2, T, IN]
        uc = up.reshape(BLOCAL, NK, L, IN)            # t = k*L + j
        u_jk = np.ascontiguousarray(
            uc.transpose(3, 2, 0, 1).reshape(IN, L * NKB)
        ).astype(np.float16)                          # col = j*NKB + b*NK + k
        in_maps.append({"blob": np.concatenate([u_jk, shared], axis=1)})

    res = bass_utils.run_bass_kernel_spmd(nc, in_maps, core_ids=list(range(NCORES)))
    LAST_RESULT = res

    y = np.empty((BATCH, T, OUT), dtype=np.float32)
    for c in range(NCORES):
        yd = res.results[c]["y"]                      # [OUT, L*NKB]
        y[BLOCAL * c:BLOCAL * (c + 1)] = (
            yd.reshape(OUT, L, BLOCAL, NK).transpose(2, 3, 1, 0)
            .reshape(BLOCAL, T, OUT)
        )
    y += np.asarray(output_bias, dtype=np.float32)
    return y


# revision 21
# speedup vs baseline: 1.0866x; 1.0866x over previous
"""Trainium2 Bass kernel for nn_BaseLinearSSM (chunked formulation).

y[b,t] = Re(C @ x_{t+1}) + D @ u[b,t] + bias,  x_{t+1} = A x_t + B u_t  (complex A,B,C)

Strategy (chunk length L=8, NK=T/L=256 chunks):
  Host (fp64): eigendecompose A = V diag(w) V^-1, Bt = V^-1 B, Ct = C V.
  Precompute:
    Pt_j = diag(w^(L-1-j)) Bt          [N,IN]  (chunk input aggregation)
    Qt_j = Ct diag(w^(j+1))            [OUT,N] (chunk boundary -> outputs)
    K_d  = Re(C A^d B), K_0 += D       [OUT,IN] real (within-chunk causal conv)
  Device (per core, batch-sharded 2 of 16; fp16 data, fp32 PSUM/scan state):
    phase 1: vt_k = sum_j Pt_j u_{kL+j}                    (matmuls, PSUM)
    phase 2: S_k = w^L S_{k-1} + vt_k  via modulate/scan/demodulate on the
             CHUNK axis only (T/L columns -> 1/8 the DVE work of a full scan);
             demod written with a one-chunk shift so S_shift[k] = beta_k =
             state at chunk start (col k=0 memset to 0 per batch element)
    phase 3: y_{kL+j} = Re(Qt_j beta_k) + sum_d K_d u_{kL+j-d}  (matmuls)
  Time is laid out (j, b, k) so every matmul has 512 contiguous columns.
  Phase 3 runs in two waves (j0..5, j6..7) with the boundary matmuls ordered
  m-outer, so the tensor engine only needs the last S tiles at the very end
  of wave A (phase-2 tail hidden behind conv + earlier-m matmuls).
  Input DMA is split over the two HWDGE rings (sync + scalar queues).
  Host shards u, permutes layouts, gathers y, adds bias.
"""

import sys

import numpy as np

if "/opt/trn_rl_repo" not in sys.path:
    sys.path.insert(0, "/opt/trn_rl_repo")

BATCH, T, IN, OUT, N = 16, 2048, 128, 128, 512
NCORES = 8
BLOCAL = BATCH // NCORES   # 2
L = 8                      # chunk length
NK = T // L                # 256 chunks per batch element
NKB = BLOCAL * NK          # 512 chunk-columns per core (b-major)
NT = N // 128              # 4 partition tiles over the state dim
COLS = BLOCAL * T          # 4096

# blob (fp16) layout / DMA piece order:
#   sync queue:   u | Pt0 | Pt1 | Pt2 | Pt3
#   scalar queue: K | tr0 | tr1 | tr2 | tr3 | (deferred) Qt
# with tr_m = ck2 | sk2 | rho2.  Qt's dma_start is issued mid-phase-1 so its
# 2 MB does not steal HBM bandwidth from the phase-1-critical pieces.
UW = L * NKB               # 4096
KW = L * 128               # 1024
PW = 2 * L * 128           # 2048 per m
TW = 2 * NKB               # 1024 per m (cos+sin)
RW = NKB                   # 512 per m (rho, col NK zeroed)
QW = L * 2 * NT * 128      # 8192
TRW = TW + RW              # 1536 per m
W16 = UW + NT * PW + KW + NT * TRW + QW  # 27648

LAST_RESULT = None
_NC_CACHE = None


def _build_nc():
    from concourse import bass, mybir
    from concourse import tile

    f32 = mybir.dt.float32
    f16 = mybir.dt.float16
    op = mybir.AluOpType

    nc = bass.Bass("TRN2", target_bir_lowering=False, debug=False)

    blob = nc.dram_tensor("blob", [128, W16], f16, kind="ExternalInput")
    yout = nc.dram_tensor("y", [OUT, COLS], f16, kind="ExternalOutput")

    with tile.TileContext(nc) as tc:
        with (
            tc.tile_pool(name="const", bufs=1) as cpool,
            tc.tile_pool(name="vsb", bufs=2) as vpool,
            tc.tile_pool(name="tmp", bufs=2) as tpool,
            tc.tile_pool(name="gz", bufs=2) as gpool,
            tc.tile_pool(name="S", bufs=1) as spool,
            tc.tile_pool(name="ysb", bufs=4) as ypool_sb,
            tc.tile_pool(name="ps", bufs=1, space="PSUM") as pspool,
        ):
            b16 = cpool.tile([128, W16], f16)
            o = [0]

            def take(w):
                s = b16[:, o[0]:o[0] + w]
                o[0] += w
                return s

            u_jk = take(UW)
            ptT = [[[None] * L for _ in range(2)] for _ in range(NT)]
            for m in range(NT):
                for ri in range(2):
                    for j in range(L):
                        ptT[m][ri][j] = take(128)
            ktT = [take(128) for _ in range(L)]
            ck2 = [None] * NT
            sk2 = [None] * NT
            rho2 = [None] * NT
            for m in range(NT):
                ck2[m] = take(NKB)
                sk2[m] = take(NKB)
                rho2[m] = take(NKB)
            qtT = [[[None] * NT for _ in range(2)] for _ in range(L)]
            for j in range(L):
                for ri in range(2):
                    for m in range(NT):
                        qtT[j][ri][m] = take(128)
            assert o[0] == W16

            # sync queue: u, then each Pt piece (phase-1 critical path)
            nc.sync.dma_start(b16[:, 0:UW], blob[:, 0:UW])
            for m in range(NT):
                lo, hi = UW + m * PW, UW + (m + 1) * PW
                nc.sync.dma_start(b16[:, lo:hi], blob[:, lo:hi])
            # scalar queue: K and the first table piece up front; the rest
            # is triggered mid-phase-1 (staggered) so the phase-1-critical
            # Pt pieces get the HBM bandwidth first
            a = UW + NT * PW
            nc.scalar.dma_start(b16[:, a:a + KW], blob[:, a:a + KW])

            def tr_dma(m):
                lo, hi = a + KW + m * TRW, a + KW + (m + 1) * TRW
                nc.scalar.dma_start(b16[:, lo:hi], blob[:, lo:hi])

            tr_dma(0)

            # PE warm-up: ~12 dependency-free matmuls on scratch run during
            # the DMA head, flipping the HAM clock gate to 8/8 (2.4 GHz)
            # before phase 1 issues. Output is discarded.
            wsc = cpool.tile([128, NKB], f16)
            nc.vector.memset(wsc[:], 0.0)
            wp = pspool.tile([128, NKB], f32, tag="vt0", bufs=1, name="warm")
            for wi in range(12):
                nc.tensor.matmul(wp[:], wsc[:, :128], wsc[:],
                                 start=(wi == 0), stop=(wi == 11))

            Sr_t = [None] * NT
            Si_t = [None] * NT
            v_t = [None] * NT

            def phase1(m):
                v_sb = [None, None]
                for ri in range(2):
                    vt = pspool.tile([128, NKB], f32, tag=f"vt{ri}", bufs=1,
                                     name=f"vt{ri}")
                    for j in range(L):
                        nc.tensor.matmul(
                            vt[:], ptT[m][ri][j], u_jk[:, j * NKB:(j + 1) * NKB],
                            start=(j == 0), stop=(j == L - 1),
                        )
                    v_sb[ri] = vpool.tile([128, NKB], f16, tag=f"v{ri}m{m}",
                                          name=f"v{ri}m{m}", bufs=1)
                    nc.scalar.copy(v_sb[ri][:], vt[:])
                    if m < NT - 1 and ri == 1:
                        tr_dma(m + 1)  # next m's tables
                if m == 2:
                    # deferred: Qt's 2 MB rides the scalar queue only after
                    # the phase-1-critical DMA pieces have landed
                    nc.scalar.dma_start(b16[:, W16 - QW:W16],
                                        blob[:, W16 - QW:W16])
                v_t[m] = v_sb

            # phase 2 for a pair of m's with cross-m interleaved issue: the
            # in-order DVE fills one chain's dependency bubbles with the
            # sibling chain's ops. GpSimd carries the imag-modulate for all
            # but the last m (whose chain gates the final boundary matmuls).
            def phase2(m):
                tl = {}
                for nm in ("t1", "t2", "t3", "t4", "t5", "t6", "t7", "t8"):
                    tl[nm] = tpool.tile([128, NKB], f16, tag=f"{nm}_{m}",
                                        name=f"{nm}_{m}", bufs=1)
                for nm in ("gr", "gi", "zr", "zi"):
                    tl[nm] = gpool.tile([128, NKB], f16, tag=f"{nm}_{m}",
                                        name=f"{nm}_{m}", bufs=1)
                vr, vi = v_t[m]
                ie = nc.vector if m == NT - 1 else nc.gpsimd
                se = nc.vector if m >= NT - 2 else nc.gpsimd
                nc.vector.tensor_tensor(tl["t1"][:], ck2[m], vr[:], op=op.mult)
                ie.tensor_tensor(tl["t3"][:], ck2[m], vi[:], op=op.mult)
                nc.vector.tensor_tensor(tl["t2"][:], sk2[m], vi[:], op=op.mult)
                ie.tensor_tensor(tl["t4"][:], sk2[m], vr[:], op=op.mult)
                nc.vector.tensor_tensor(tl["gr"][:], tl["t1"][:],
                                        tl["t2"][:], op=op.add)
                ie.tensor_tensor(tl["gi"][:], tl["t3"][:],
                                 tl["t4"][:], op=op.subtract)
                nc.vector.tensor_tensor_scan(
                    tl["zr"][:], rho2[m], tl["gr"][:], 0.0,
                    op0=op.mult, op1=op.add)
                nc.vector.tensor_tensor_scan(
                    tl["zi"][:], rho2[m], tl["gi"][:], 0.0,
                    op0=op.mult, op1=op.add)
                nc.vector.tensor_tensor(tl["t5"][:], ck2[m],
                                        tl["zr"][:], op=op.mult)
                nc.vector.tensor_tensor(tl["t6"][:], sk2[m],
                                        tl["zi"][:], op=op.mult)
                nc.vector.tensor_tensor(tl["t7"][:], sk2[m],
                                        tl["zr"][:], op=op.mult)
                nc.vector.tensor_tensor(tl["t8"][:], ck2[m],
                                        tl["zi"][:], op=op.mult)
                Sr = spool.tile([128, NKB], f16, tag=f"Sr{m}", name=f"Sr{m}")
                Si = spool.tile([128, NKB], f16, tag=f"Si{m}", name=f"Si{m}")
                nc.gpsimd.memset(Sr[:, 0:1], 0.0)
                nc.gpsimd.memset(Sr[:, NK:NK + 1], 0.0)
                nc.gpsimd.memset(Si[:, 0:1], 0.0)
                nc.gpsimd.memset(Si[:, NK:NK + 1], 0.0)
                for b in range(BLOCAL):
                    a0 = b * NK
                    nc.vector.tensor_tensor(
                        Sr[:, a0 + 1:a0 + NK], tl["t5"][:, a0:a0 + NK - 1],
                        tl["t6"][:, a0:a0 + NK - 1], op=op.subtract,
                    )
                    se.tensor_tensor(
                        Si[:, a0 + 1:a0 + NK], tl["t7"][:, a0:a0 + NK - 1],
                        tl["t8"][:, a0:a0 + NK - 1], op=op.add,
                    )
                Sr_t[m], Si_t[m] = Sr, Si

            for m in range(NT):
                phase1(m)
                phase2(m)

            # phase 3, two waves (j0..3, j4..7): conv first (only needs u/K),
            # then boundary matmuls m-outer so S(m) is consumed in completion
            # order. convB for j4/j5 is issued before wave A's m=3 batch to
            # keep the PE streaming while the last S tiles finish.
            yps = {}

            def conv(j):
                yps[j] = pspool.tile([128, NKB], f32, tag="y", bufs=6,
                                     name=f"y{j}")
                for d in range(j + 1):
                    nc.tensor.matmul(
                        yps[j][:], ktT[d],
                        u_jk[:, (j - d) * NKB:(j - d + 1) * NKB],
                        start=(d == 0), stop=False, skip_group_check=True,
                    )

            def bnd(j, m):
                nc.tensor.matmul(
                    yps[j][:], qtT[j][0][m], Sr_t[m][:], start=False,
                    stop=False, skip_group_check=True,
                )
                nc.tensor.matmul(
                    yps[j][:], qtT[j][1][m], Si_t[m][:], start=False,
                    stop=(m == NT - 1), skip_group_check=True,
                )

            def evict(j, eng):
                ysb = ypool_sb.tile([128, NKB], f16, tag="ysb")
                eng.copy(ysb[:], yps[j][:]) if eng is nc.scalar else \
                    eng.tensor_copy(ysb[:], yps[j][:])
                nc.sync.dma_start(yout[:, j * NKB:(j + 1) * NKB], ysb[:])

            for j in range(4):
                conv(j)
            for m in range(NT - 1):
                for j in range(4):
                    bnd(j, m)
            conv(4)
            conv(5)
            for j in range(4):
                bnd(j, NT - 1)
            for j in range(4):
                evict(j, nc.scalar)
            conv(6)
            conv(7)
            for m in range(NT):
                for j in range(4, L):
                    bnd(j, m)
            evict(4, nc.scalar)
            evict(5, nc.scalar)
            evict(6, nc.scalar)
            evict(7, nc.vector)

    _legalize_multi_waits(nc)
    return nc


def _legalize_multi_waits(nc):
    """This walrus build accepts a single sync wait per instruction; split
    any multi-wait instruction into same-engine single-wait NoOps + the
    original carrying the last wait (program order chains them)."""
    import bass_rust
    from concourse import mybir

    uid = [0]
    for fn in nc.m.functions:
        for bb in fn.blocks:
            insts = bb.instructions
            new = []
            changed = False
            for inst in insts:
                si = inst.sync_info
                if si is not None and len(si.on_wait) > 1:
                    waits = list(si.on_wait)
                    for w in waits[:-1]:
                        uid[0] += 1
                        new.append(mybir.InstNoOp(
                            name=f"mwsplit-{uid[0]}",
                            engine=inst.engine,
                            ins=[], outs=[],
                            sync_info=bass_rust.SyncInfo(on_wait=[w], on_update=[]),
                        ))
                    inst.sync_info = bass_rust.SyncInfo(
                        on_wait=[waits[-1]], on_update=list(si.on_update)
                    )
                    changed = True
                new.append(inst)
            if changed:
                bb.instructions = new


def _host_prep(A_re, A_im, B_re, B_im, C_re, C_im, D_w):
    """fp64 eigendecomposition + chunked-formulation weight/table layouts.
    Returns shared fp16 tail of the blob: [128, W16 - UW]."""
    A = A_re.astype(np.float64) + 1j * A_im.astype(np.float64)
    w, V = np.linalg.eig(A)
    Vinv = np.linalg.inv(V)
    Bt = Vinv @ (B_re.astype(np.float64) + 1j * B_im.astype(np.float64))
    Ct = (C_re.astype(np.float64) + 1j * C_im.astype(np.float64)) @ V

    Pt = np.stack([(w ** (L - 1 - j))[:, None] * Bt for j in range(L)])  # [L,N,IN]
    Qt = np.stack([Ct * (w ** (j + 1))[None, :] for j in range(L)])      # [L,OUT,N]
    K = np.empty((L, OUT, IN))
    Ad = np.eye(N, dtype=complex)
    Bc = B_re.astype(np.float64) + 1j * B_im.astype(np.float64)
    Cc = C_re.astype(np.float64) + 1j * C_im.astype(np.float64)
    for d in range(L):
        K[d] = (Cc @ Ad @ Bc).real
        Ad = A @ Ad
    K[0] += D_w.astype(np.float64)

    wL = w ** L
    rhoL = np.abs(wL)
    phi = np.angle(wL)
    kk = np.arange(NK)
    cosk = np.cos(np.outer(phi, kk + 1))  # [N, NK]
    sink = np.sin(np.outer(phi, kk + 1))

    parts = []
    for m in range(NT):
        sl = slice(m * 128, (m + 1) * 128)
        for Pp in (Pt.real, Pt.imag):
            for j in range(L):
                parts.append(np.ascontiguousarray(Pp[j].T[:, sl]))  # [IN, 128]
    for d in range(L):
        parts.append(np.ascontiguousarray(K[d].T))  # [IN, OUT]
    for m in range(NT):
        sl = slice(m * 128, (m + 1) * 128)
        parts.append(np.tile(cosk[sl], (1, BLOCAL)))  # [128, NKB]
        parts.append(np.tile(sink[sl], (1, BLOCAL)))
        rb = np.broadcast_to(rhoL[sl][:, None], (128, NKB)).copy()
        rb[:, NK] = 0.0  # reset scan state at second batch element
        parts.append(rb)
    for j in range(L):
        for Qp in (Qt[j].real, -Qt[j].imag):
            QT = np.ascontiguousarray(Qp.T)  # [N, OUT]
            for m in range(NT):
                parts.append(QT[m * 128:(m + 1) * 128])
    shared = np.concatenate(parts, axis=1).astype(np.float16)
    assert shared.shape == (128, W16 - UW)
    return shared


def _ensure_axon_hooks():
    """Provide antenv.axon_hooks if the image lacks it (needed only for
    trace=True NTFF profiling; run path works without)."""
    import types
    try:
        from antenv import axon_hooks  # noqa: F401
        return
    except ImportError:
        pass
    try:
        import antenv
        mod = types.ModuleType("antenv.axon_hooks")
        _hook = [None]
        mod.set_axon_ntff_profile_hook = lambda h: _hook.__setitem__(0, h)
        mod.get_axon_ntff_profile_hook = lambda: _hook[0]
        sys.modules["antenv.axon_hooks"] = mod
        antenv.axon_hooks = mod
        if "/root/.axon_site" not in sys.path:
            sys.path.insert(0, "/root/.axon_site")
        from trn_agent_boot.trn_boot import _ntff_profile_via_ctypes
        h = _ntff_profile_via_ctypes("/opt/axon/libaxon_pjrt.so")
        if h is not None:
            mod.set_axon_ntff_profile_hook(h)
    except Exception:
        pass


def kernel(u, A_re, A_im, B_re, B_im, C_re, C_im, D_w, output_bias):
    global LAST_RESULT, _NC_CACHE
    from concourse import bass_utils

    _ensure_axon_hooks()

    u = np.asarray(u, dtype=np.float32)
    shared = _host_prep(
        np.asarray(A_re), np.asarray(A_im), np.asarray(B_re), np.asarray(B_im),
        np.asarray(C_re), np.asarray(C_im), np.asarray(D_w)
    )

    if _NC_CACHE is None:
        _NC_CACHE = _build_nc()
    nc = _NC_CACHE

    in_maps = []
    for c in range(NCORES):
        up = u[BLOCAL * c:BLOCAL * (c + 1)]           # [2, T, IN]
        uc = up.reshape(BLOCAL, NK, L, IN)            # t = k*L + j
        u_jk = np.ascontiguousarray(
            uc.transpose(3, 2, 0, 1).reshape(IN, L * NKB)
        ).astype(np.float16)                          # col = j*NKB + b*NK + k
        in_maps.append({"blob": np.concatenate([u_jk, shared], axis=1)})

    res = bass_utils.run_bass_kernel_spmd(nc, in_maps, core_ids=list(range(NCORES)))
    LAST_RESULT = res

    y = np.empty((BATCH, T, OUT), dtype=np.float32)
    for c in range(NCORES):
        yd = np.asarray(res.results[c]["y"], dtype=np.float32)  # [OUT, L*NKB]
        y[BLOCAL * c:BLOCAL * (c + 1)] = (
            yd.reshape(OUT, L, BLOCAL, NK).transpose(2, 3, 1, 0)
            .reshape(BLOCAL, T, OUT)
        )
    y += np.asarray(output_bias, dtype=np.float32)
    return y
